# revision 1
# baseline (speedup 1.0000x reference)
"""Global-KNN GCN kernel for Trainium2 (8 NeuronCores, SPMD).

Heavy part (161 GFLOP pairwise-score matmul + top-9 per row) runs on
device, row-sharded 784 rows/core. Scores s_ij = x_i.x_j - 0.5*||x_j||^2
rank identically to -squared-distance. Top-9 largest per row via the DVE
max8 / max_index / match_replace instruction triple, done per DB half so
the fp32 score slab fits SBUF. Host does the cheap part: edge list, sym
norm, two sparse aggregations and the two small dense layers.
"""

import os
import numpy as np

B, H, W, C = 32, 14, 14, 2048
N = B * H * W            # 6272 nodes
K = 8                    # neighbors (excluding self)
N_CORES = 8
ROWS = N // N_CORES      # 784 rows per core
MT, MP = 7, 112          # 7 partition tiles of 112 rows = 784
HALF = N // 2            # 3136 columns per half-slab
NB = 448                 # psum tile free size (one bank; 3136 = 7*448)
NCH = HALF // NB         # 7 column chunks per half
KCH = C // 128           # 16 contraction chunks

LAST_EXEC_NS = None
LAST_KNN = None
_PROG = None


def _build_program():
    from concourse import bacc, tile, mybir

    f32 = mybir.dt.float32
    f32r = mybir.dt.float32r
    u32 = mybir.dt.uint32

    nc = bacc.Bacc("TRN2", target_bir_lowering=False)
    xfT = nc.declare_dram_parameter("xfT", [C, N], f32r, isOutput=False)
    xrT = nc.declare_dram_parameter("xrT", [C, ROWS], f32r, isOutput=False)
    nhsq = nc.declare_dram_parameter("nhsq", [2, N], f32r, isOutput=False)
    vals = nc.declare_dram_parameter("vals", [14, ROWS, 16], f32, isOutput=True)
    idxs = nc.declare_dram_parameter("idxs", [14, ROWS, 16], u32, isOutput=True)

    Act = mybir.ActivationFunctionType

    with tile.TileContext(nc) as tc:
        with (
            tc.tile_pool(name="persist", bufs=1) as pp,
            tc.tile_pool(name="rhs", bufs=4) as rp,
            tc.tile_pool(name="chunk", bufs=6) as cp,
            tc.tile_pool(name="small", bufs=8) as kp,
            tc.tile_pool(name="psum", bufs=8, space="PSUM") as psp,
        ):
            # own row block, transposed: [C, ROWS] laid out as MT*KCH
            # stationary [128, MP] panels side by side
            lhsT = pp.tile([128, MT * KCH * MP], f32r)
            for t in range(MT):
                for ki in range(KCH):
                    off = (t * KCH + ki) * MP
                    nc.sync.dma_start(
                        out=lhsT[:, off:off + MP],
                        in_=xrT[ki * 128:(ki + 1) * 128, t * MP:(t + 1) * MP],
                    )
            nh = pp.tile([1, N], f32r)
            nc.sync.dma_start(out=nh[:], in_=nhsq[0:1, :])
            ones = pp.tile([1, MP], f32r)
            nc.sync.dma_start(out=ones[:], in_=nhsq[1:2, 0:MP])

            for j in range(N // NB):            # 14 column chunks
                c0 = j * NB
                pss = [psp.tile([MP, NB], f32, tag="ps", name=f"ps_{j}_{t}") for t in range(MT)]
                for ki in range(KCH):
                    rhs = rp.tile([128, NB], f32r, tag="rhs")
                    nc.sync.dma_start(
                        out=rhs[:],
                        in_=xfT[ki * 128:(ki + 1) * 128, c0:c0 + NB],
                    )
                    for t in range(MT):
                        off = (t * KCH + ki) * MP
                        nc.tensor.matmul(
                            pss[t][:], lhsT[:, off:off + MP], rhs[:],
                            start=(ki == 0), stop=False, skip_group_check=True,
                        )
                for t in range(MT):
                    # += -0.5*||x_j||^2 broadcast down partitions
                    nc.tensor.matmul(
                        pss[t][:], ones[:, :], nh[:, c0:c0 + NB],
                        start=False, stop=True, skip_group_check=True,
                    )
                    cb = cp.tile([MP, NB], f32, tag="cb")
                    nc.scalar.activation(cb[:], pss[t][:], Act.Copy)
                    v1 = kp.tile([MP, 8], f32, tag="v1")
                    i1 = kp.tile([MP, 8], u32, tag="i1")
                    v2 = kp.tile([MP, 8], f32, tag="v2")
                    i2 = kp.tile([MP, 8], u32, tag="i2")
                    rep = cp.tile([MP, NB], f32, tag="rep")
                    vo = kp.tile([MP, 16], f32, tag="vo")
                    io = kp.tile([MP, 16], u32, tag="io")
                    nc.vector.max(v1[:], cb[:])
                    nc.vector.max_index(i1[:], v1[:], cb[:])
                    nc.vector.match_replace(rep[:], v1[:], cb[:], -3.0e38)
                    nc.vector.max(v2[:], rep[:])
                    nc.vector.max_index(i2[:], v2[:], rep[:])
                    nc.vector.tensor_copy(vo[:, 0:8], v1[:])
                    nc.vector.tensor_copy(vo[:, 8:16], v2[:])
                    nc.vector.tensor_copy(io[:, 0:8], i1[:])
                    nc.vector.tensor_copy(io[:, 8:16], i2[:])
                    r0, r1 = t * MP, (t + 1) * MP
                    nc.sync.dma_start(out=vals[j, r0:r1, :], in_=vo[:])
                    nc.sync.dma_start(out=idxs[j, r0:r1, :], in_=io[:])
    nc.compile()
    return nc


def _knn_from_device(x_flat):
    """Run the SPMD program; return knn [N, K] int64 global indices."""
    global LAST_EXEC_NS, _PROG
    from concourse.bass_utils import run_bass_kernel_spmd

    if _PROG is None:
        _PROG = _build_program()

    xfT = np.ascontiguousarray(x_flat.T)                     # [C, N]
    nhsq = np.ones((2, N), dtype=np.float32)
    nhsq[0] = -0.5 * np.sum(x_flat * x_flat, axis=1, dtype=np.float32)
    in_maps = []
    for c in range(N_CORES):
        in_maps.append({
            "xfT": xfT,
            "xrT": np.ascontiguousarray(xfT[:, c * ROWS:(c + 1) * ROWS]),
            "nhsq": nhsq,
        })
    res = run_bass_kernel_spmd(
        _PROG, in_maps, list(range(N_CORES)),
        trace=bool(os.environ.get("KNN_TRACE")),
    )
    if res.exec_time_ns is not None:
        LAST_EXEC_NS = res.exec_time_ns

    # per-core outputs are [14, ROWS, 16] -> [ROWS, 224]
    vals_all = np.concatenate(
        [r["vals"].transpose(1, 0, 2).reshape(ROWS, 224) for r in res.results], axis=0)
    loc = np.concatenate(
        [r["idxs"].transpose(1, 0, 2).reshape(ROWS, 224) for r in res.results],
        axis=0).astype(np.int64)
    idxs_all = loc + (np.arange(14, dtype=np.int64) * NB).repeat(16)[None, :]

    # coarse top-32 by device (float32r) score, then exact fp32 re-score
    part = np.argpartition(-vals_all, 32, axis=1)[:, :32]
    idxs_all = np.take_along_axis(idxs_all, part, axis=1)    # [N, 32]
    sq = np.sum(x_flat * x_flat, axis=1, dtype=np.float32)
    exact = np.empty((N, 32), dtype=np.float32)
    BLK = 196
    for r0 in range(0, N, BLK):
        r1 = r0 + BLK
        cand = idxs_all[r0:r1]                               # [b, 32]
        xc = x_flat[cand]                                    # [b, 32, C]
        exact[r0:r1] = np.einsum("bc,bkc->bk", x_flat[r0:r1], xc,
                                 dtype=np.float32) - 0.5 * sq[cand]
    order = np.argsort(-exact, axis=1, kind="stable")[:, :K + 1]
    top = np.take_along_axis(idxs_all, order, axis=1)        # [N, 9]
    rows = np.arange(N)[:, None]
    selfpos = top == rows
    has_self = selfpos.any(axis=1)
    rem = np.where(has_self, selfpos.argmax(axis=1), K)      # drop self, else 9th
    keep = np.ones((N, K + 1), dtype=bool)
    keep[np.arange(N), rem] = False
    global LAST_KNN
    LAST_KNN = top[keep].reshape(N, K)
    return LAST_KNN


def kernel(x, W1, b1, W2, b2):
    x = np.asarray(x, dtype=np.float32)
    W1 = np.asarray(W1, dtype=np.float32)
    b1 = np.asarray(b1, dtype=np.float32)
    W2 = np.asarray(W2, dtype=np.float32)
    b2 = np.asarray(b2, dtype=np.float32)

    xf = x.reshape(N, C)
    knn = _knn_from_device(xf)

    src = np.repeat(np.arange(N, dtype=np.int64), K)
    dst = knn.reshape(-1)
    loops = np.arange(N, dtype=np.int64)
    src = np.concatenate([src, loops])
    dst = np.concatenate([dst, loops])

    deg = np.bincount(dst, minlength=N).astype(np.float32)
    dinv = 1.0 / np.sqrt(np.maximum(deg, 1.0))
    norm = (dinv[src] * dinv[dst]).astype(np.float32)

    try:
        import scipy.sparse as sps
        A = sps.csr_matrix((norm, (dst, src)), shape=(N, N), dtype=np.float32)

        def agg(hw):
            return A @ hw
    except Exception:
        def agg(hw):
            out = np.zeros_like(hw)
            np.add.at(out, dst, hw[src] * norm[:, None])
            return out

    h1 = np.maximum(agg(xf @ W1) + b1, 0.0).astype(np.float32)
    h2 = np.maximum(agg(h1 @ W2) + b2, 0.0).astype(np.float32)
    return h2.reshape(B, H, W, W2.shape[1]).astype(np.float32)



# revision 3
# speedup vs baseline: 2.2033x; 2.2033x over previous
"""Global-KNN GCN kernel for Trainium2 (8 NeuronCores, SPMD).

Device computes the 161-GFLOP pairwise score matrix in fp8 (e4m3,
DoubleRow perf mode: 256-deep contraction per matmul) plus a bf16 rank-1
matmul adding the centered -0.5*||x_j||^2 term, then per-448-column-chunk
top-8 (values + indices) on the DVE over bf16 scores. All of x stays
SBUF-resident in fp8 (loaded once, ~12.8 MB/core). Each core's input is
rotated so its own 784-row block sits at column 0; the stationary row
panels are slices of the same resident tensor.

Host does the cheap O(N*K) part: merges the 14x8 per-chunk candidates,
exact fp32 re-score of the top-64, builds the KNN edge list, and runs the
two small GCN layers (sparse aggregation + dense matmuls).
"""

import os
import numpy as np

B, H, W, C = 32, 14, 14, 2048
N = B * H * W            # 6272 nodes
K = 8                    # neighbors (excluding self)
N_CORES = 8
ROWS = N // N_CORES      # 784 rows per core
RT = 112                 # rows per tile
NT = ROWS // RT          # 7 row tiles
NB = 448                 # column chunk (psum free size)
NCH = N // NB            # 14 column chunks
KP = C // 256            # 8 channel pair-chunks (256 channels each)
CAND = NCH * 8           # 112 candidates per row
TOPC = 64                # host exact re-score depth

LAST_EXEC_NS = None
LAST_KNN = None
_PROG = None


def _build_program():
    from concourse import bacc, tile, mybir

    f32 = mybir.dt.float32
    bf16 = mybir.dt.bfloat16
    f8 = mybir.dt.float8e4
    u16 = mybir.dt.uint16

    nc = bacc.Bacc("TRN2", target_bir_lowering=False)
    x8 = nc.declare_dram_parameter("x8", [KP, 128, 2, N], f8, isOutput=False)
    aux = nc.declare_dram_parameter("aux", [2, N], bf16, isOutput=False)
    cand = nc.declare_dram_parameter("cand", [NT, 2, RT, 112], u16, isOutput=True)

    Act = mybir.ActivationFunctionType
    DR = mybir.MatmulPerfMode.DoubleRow

    with tile.TileContext(nc) as tc:
        with (
            tc.tile_pool(name="persist", bufs=1) as pp,
            tc.tile_pool(name="score", bufs=6) as cp,
            tc.tile_pool(name="stage", bufs=4) as sp,
            tc.tile_pool(name="psum", bufs=8, space="PSUM") as psp,
        ):
            xs = []
            for kp in range(KP):
                t = pp.tile([128, 2, N], f8, name=f"xs{kp}")
                nc.sync.dma_start(out=t[:], in_=x8[kp])
                xs.append(t)
            nhc = pp.tile([1, N], bf16)
            nc.sync.dma_start(out=nhc[:], in_=aux[0:1, :])
            ones = pp.tile([1, RT], bf16)
            nc.sync.dma_start(out=ones[:], in_=aux[1:2, 0:RT])

            for t in range(NT):
                r0 = t * RT
                for jb in range(2):
                    pss = [
                        psp.tile([RT, NB], f32, tag="ps", name=f"ps_{t}_{jb}_{jp}")
                        for jp in range(7)
                    ]
                    for kp in range(KP):
                        for jp in range(7):
                            j = jb * 7 + jp
                            nc.tensor.matmul(
                                pss[jp][:, :],
                                xs[kp][:, :, r0:r0 + RT],
                                xs[kp][:, :, j * NB:(j + 1) * NB],
                                start=(kp == 0), stop=False,
                                perf_mode=DR, skip_group_check=True,
                            )
                    stage = sp.tile([RT, 112], u16, tag="st")
                    for jp in range(7):
                        j = jb * 7 + jp
                        nc.tensor.matmul(
                            pss[jp][:, :], ones[:, :],
                            nhc[:, j * NB:(j + 1) * NB],
                            start=False, stop=True, skip_group_check=True,
                        )
                        cb = cp.tile([RT, NB], bf16, tag="cb")
                        nc.scalar.activation(cb[:, :], pss[jp][:, :], Act.Copy)
                        nc.vector.max(
                            stage[:, jp * 16:jp * 16 + 8].bitcast(bf16),
                            cb[:, :])
                        nc.vector.max_index(
                            stage[:, jp * 16 + 8:jp * 16 + 16],
                            stage[:, jp * 16:jp * 16 + 8].bitcast(bf16),
                            cb[:, :])
                    nc.gpsimd.dma_start(out=cand[t, jb], in_=stage[:, :])
    nc.compile()
    return nc


def _knn_from_device(x_flat):
    """Run the SPMD program; return knn [N, K] int64 global indices."""
    global LAST_EXEC_NS, LAST_KNN, _PROG
    import ml_dtypes
    from concourse.bass_utils import run_bass_kernel_spmd

    if _PROG is None:
        _PROG = _build_program()

    xq8 = x_flat.astype(ml_dtypes.float8_e4m3)               # [N, C]
    sq = np.sum(x_flat * x_flat, axis=1, dtype=np.float32)
    nhc = (-0.5 * (sq - sq.mean())).astype(ml_dtypes.bfloat16)
    ones = np.ones((N,), dtype=ml_dtypes.bfloat16)
    # x8 layout [kp, p, i, n]: channel = kp*256 + i*128 + p
    x8T = np.ascontiguousarray(xq8.T)                        # [C, N]
    x8 = np.ascontiguousarray(
        x8T.reshape(KP, 2, 128, N).transpose(0, 2, 1, 3))    # [kp, p, i, n]

    in_maps = []
    for c in range(N_CORES):
        sh = c * ROWS
        in_maps.append({
            "x8": np.ascontiguousarray(np.roll(x8, -sh, axis=3)),
            "aux": np.ascontiguousarray(np.stack([np.roll(nhc, -sh), ones])),
        })
    res = run_bass_kernel_spmd(
        _PROG, in_maps, list(range(N_CORES)),
        trace=bool(os.environ.get("KNN_TRACE")),
    )
    if res.exec_time_ns is not None:
        LAST_EXEC_NS = res.exec_time_ns

    vals = np.empty((N, CAND), dtype=np.float32)
    cidx = np.empty((N, CAND), dtype=np.int64)
    jbase = (np.arange(NCH, dtype=np.int64) * NB)[None, :, None]
    for c, r in enumerate(res.results):
        o = r["cand"].reshape(NT, 2, RT, 7, 16)
        o = o.transpose(0, 2, 1, 3, 4).reshape(ROWS, NCH, 16)
        v = np.ascontiguousarray(o[:, :, 0:8]).view(ml_dtypes.bfloat16)
        loc = o[:, :, 8:16].astype(np.int64)
        gcol = (jbase + loc + c * ROWS) % N
        vals[c * ROWS:(c + 1) * ROWS] = v.astype(np.float32).reshape(ROWS, CAND)
        cidx[c * ROWS:(c + 1) * ROWS] = gcol.reshape(ROWS, CAND)

    # coarse top-TOPC by device score, then exact fp32 re-score
    part = np.argpartition(-vals, TOPC, axis=1)[:, :TOPC]
    cidx = np.take_along_axis(cidx, part, axis=1)            # [N, TOPC]
    exact = np.empty((N, TOPC), dtype=np.float32)
    BLK = 196
    for r0 in range(0, N, BLK):
        r1 = r0 + BLK
        cn = cidx[r0:r1]                                     # [b, TOPC]
        xc = x_flat[cn]                                      # [b, TOPC, C]
        exact[r0:r1] = np.einsum("bc,bkc->bk", x_flat[r0:r1], xc,
                                 dtype=np.float32) - 0.5 * sq[cn]
    order = np.argsort(-exact, axis=1, kind="stable")[:, :K + 1]
    top = np.take_along_axis(cidx, order, axis=1)            # [N, 9]
    rows = np.arange(N)[:, None]
    selfpos = top == rows
    has_self = selfpos.any(axis=1)
    rem = np.where(has_self, selfpos.argmax(axis=1), K)      # drop self, else 9th
    keep = np.ones((N, K + 1), dtype=bool)
    keep[np.arange(N), rem] = False
    LAST_KNN = top[keep].reshape(N, K)
    return LAST_KNN


def kernel(x, W1, b1, W2, b2):
    x = np.asarray(x, dtype=np.float32)
    W1 = np.asarray(W1, dtype=np.float32)
    b1 = np.asarray(b1, dtype=np.float32)
    W2 = np.asarray(W2, dtype=np.float32)
    b2 = np.asarray(b2, dtype=np.float32)

    xf = x.reshape(N, C)
    knn = _knn_from_device(xf)

    src = np.repeat(np.arange(N, dtype=np.int64), K)
    dst = knn.reshape(-1)
    loops = np.arange(N, dtype=np.int64)
    src = np.concatenate([src, loops])
    dst = np.concatenate([dst, loops])

    deg = np.bincount(dst, minlength=N).astype(np.float32)
    dinv = 1.0 / np.sqrt(np.maximum(deg, 1.0))
    norm = (dinv[src] * dinv[dst]).astype(np.float32)

    try:
        import scipy.sparse as sps
        A = sps.csr_matrix((norm, (dst, src)), shape=(N, N), dtype=np.float32)

        def agg(hw):
            return A @ hw
    except Exception:
        def agg(hw):
            out = np.zeros_like(hw)
            np.add.at(out, dst, hw[src] * norm[:, None])
            return out

    h1 = np.maximum(agg(xf @ W1) + b1, 0.0).astype(np.float32)
    h2 = np.maximum(agg(h1 @ W2) + b2, 0.0).astype(np.float32)
    return h2.reshape(B, H, W, W2.shape[1]).astype(np.float32)


# revision 5
# speedup vs baseline: 2.6063x; 1.1829x over previous
"""Global-KNN GCN kernel for Trainium2 (8 NeuronCores, SPMD).

Device computes the 161-GFLOP pairwise score matrix in fp8 (e4m3,
DoubleRow perf mode: 256-deep contraction per matmul at ~1 cyc/col,
2x bf16 FLOP rate), then per-448-column-chunk top-8 (values + indices)
on the DVE over bf16 scores. The centered -0.5*||x_j||^2 ranking term is
folded into the contraction itself: channels 2046/2047 are sacrificed --
the stationary (row) side carries (1, 1) there and the moving (column)
side carries a coarse+residual fp8 split of the centered norm term. All
of x stays SBUF-resident in fp8 (loaded once, ~12.8 MB/core). Each
core's input is rotated so its own 784-row block sits at column 0; the
stationary row panels are slices of the same resident tensor (except the
last channel group, which has its own modified panel).

Host does the cheap O(N*K) part: merges the 14x8 per-chunk candidates,
exact fp32 re-score of the top-64, builds the KNN edge list, and runs the
two small GCN layers (sparse aggregation + dense matmuls).
"""

import os
import numpy as np

B, H, W, C = 32, 14, 14, 2048
N = B * H * W            # 6272 nodes
K = 8                    # neighbors (excluding self)
N_CORES = 8
ROWS = N // N_CORES      # 784 rows per core
RT = 112                 # rows per tile
NT = ROWS // RT          # 7 row tiles
NB = 448                 # column chunk (psum free size)
NCH = N // NB            # 14 column chunks
KP = C // 256            # 8 channel pair-chunks (256 channels each)
CAND = NCH * 8           # 112 candidates per row
TOPC = 64                # host exact re-score depth

LAST_EXEC_NS = None
LAST_KNN = None
_PROG = None


def _build_program():
    from concourse import bacc, tile, mybir

    f32 = mybir.dt.float32
    bf16 = mybir.dt.bfloat16
    f8 = mybir.dt.float8e4
    u16 = mybir.dt.uint16

    nc = bacc.Bacc("TRN2", target_bir_lowering=False)
    x8 = nc.declare_dram_parameter("x8", [KP, 128, 2, N], f8, isOutput=False)
    xr7d = nc.declare_dram_parameter("xr7", [128, 2, ROWS], f8, isOutput=False)
    cand = nc.declare_dram_parameter("cand", [NT, NCH, RT, 16], u16, isOutput=True)

    Act = mybir.ActivationFunctionType
    DR = mybir.MatmulPerfMode.DoubleRow

    with tile.TileContext(nc) as tc:
        with (
            tc.tile_pool(name="persist", bufs=1) as pp,
            tc.tile_pool(name="score", bufs=6) as cp,
            tc.tile_pool(name="stage", bufs=10) as sp,
            tc.tile_pool(name="psum", bufs=8, space="PSUM") as psp,
        ):
            xs = [pp.tile([128, 2, N], f8, name=f"xs{kp}") for kp in range(KP)]
            HALF = N // 2
            qs = [nc.sync, nc.scalar]
            for h in range(2):
                for kp in range(KP):
                    qs[(h * KP + kp) % 2].dma_start(
                        out=xs[kp][:, :, h * HALF:(h + 1) * HALF],
                        in_=x8[kp, :, :, h * HALF:(h + 1) * HALF])
            xr7 = pp.tile([128, 2, ROWS], f8)
            nc.gpsimd.dma_start(out=xr7[:], in_=xr7d[:])

            for jb in range(2):
                for t in range(NT):
                    r0 = t * RT
                    pss = [
                        psp.tile([RT, NB], f32, tag="ps", name=f"ps_{t}_{jb}_{jp}")
                        for jp in range(7)
                    ]
                    for kp in range(KP):
                        lhsT = (xs[kp][:, :, r0:r0 + RT] if kp < KP - 1
                                else xr7[:, :, r0:r0 + RT])
                        for jp in range(7):
                            j = jb * 7 + jp
                            nc.tensor.matmul(
                                pss[jp][:, :],
                                lhsT,
                                xs[kp][:, :, j * NB:(j + 1) * NB],
                                start=(kp == 0), stop=(kp == KP - 1),
                                perf_mode=DR, skip_group_check=True,
                            )
                    for jp in range(7):
                        j = jb * 7 + jp
                        cb = cp.tile([RT, NB], bf16, tag="cb")
                        nc.scalar.activation(cb[:, :], pss[jp][:, :], Act.Copy)
                        stage = sp.tile([RT, 16], u16, tag="st")
                        nc.vector.max(stage[:, 0:8].bitcast(bf16), cb[:, :])
                        nc.vector.max_index(
                            stage[:, 8:16],
                            stage[:, 0:8].bitcast(bf16),
                            cb[:, :])
                        nc.gpsimd.dma_start(out=cand[t, j], in_=stage[:, :])
    nc.compile()
    return nc


def _knn_from_device(x_flat):
    """Run the SPMD program; return knn [N, K] int64 global indices."""
    global LAST_EXEC_NS, LAST_KNN, _PROG
    import ml_dtypes
    from concourse.bass_utils import run_bass_kernel_spmd

    if _PROG is None:
        _PROG = _build_program()

    xq8 = x_flat.astype(ml_dtypes.float8_e4m3)               # [N, C]
    sq = np.sum(x_flat * x_flat, axis=1, dtype=np.float32)
    nhc = -0.5 * (sq - sq.mean())
    a = nhc.astype(ml_dtypes.float8_e4m3)
    bres = (nhc - a.astype(np.float32)).astype(ml_dtypes.float8_e4m3)
    # x8 layout [kp, p, i, n]: channel = kp*256 + i*128 + p
    x8T = np.ascontiguousarray(xq8.T)                        # [C, N]
    x8 = np.ascontiguousarray(
        x8T.reshape(KP, 2, 128, N).transpose(0, 2, 1, 3))    # [kp, p, i, n]
    # fold the norm term into sacrificed channels 2046/2047 (kp=7, i=1,
    # p=126/127): moving side carries (a, b); stationary side carries (1, 1)
    x8[KP - 1, 126, 1, :] = a
    x8[KP - 1, 127, 1, :] = bres

    one8 = np.float32(1.0).astype(ml_dtypes.float8_e4m3)
    in_maps = []
    for c in range(N_CORES):
        sh = c * ROWS
        x8c = np.ascontiguousarray(np.roll(x8, -sh, axis=3))
        xr7 = np.ascontiguousarray(x8c[KP - 1, :, :, 0:ROWS])
        xr7[126, 1, :] = one8
        xr7[127, 1, :] = one8
        in_maps.append({"x8": x8c, "xr7": xr7})
    res = run_bass_kernel_spmd(
        _PROG, in_maps, list(range(N_CORES)),
        trace=bool(os.environ.get("KNN_TRACE")),
    )
    if res.exec_time_ns is not None:
        LAST_EXEC_NS = res.exec_time_ns

    vals = np.empty((N, CAND), dtype=np.float32)
    cidx = np.empty((N, CAND), dtype=np.int64)
    jbase = (np.arange(NCH, dtype=np.int64) * NB)[None, :, None]
    for c, r in enumerate(res.results):
        o = r["cand"]                                        # [NT, NCH, RT, 16]
        o = o.transpose(0, 2, 1, 3).reshape(ROWS, NCH, 16)
        v = np.ascontiguousarray(o[:, :, 0:8]).view(ml_dtypes.bfloat16)
        loc = o[:, :, 8:16].astype(np.int64)
        gcol = (jbase + loc + c * ROWS) % N
        vals[c * ROWS:(c + 1) * ROWS] = v.astype(np.float32).reshape(ROWS, CAND)
        cidx[c * ROWS:(c + 1) * ROWS] = gcol.reshape(ROWS, CAND)

    # coarse top-TOPC by device score, then exact fp32 re-score
    part = np.argpartition(-vals, TOPC, axis=1)[:, :TOPC]
    cidx = np.take_along_axis(cidx, part, axis=1)            # [N, TOPC]
    exact = np.empty((N, TOPC), dtype=np.float32)
    BLK = 196
    for r0 in range(0, N, BLK):
        r1 = r0 + BLK
        cn = cidx[r0:r1]                                     # [b, TOPC]
        xc = x_flat[cn]                                      # [b, TOPC, C]
        exact[r0:r1] = np.einsum("bc,bkc->bk", x_flat[r0:r1], xc,
                                 dtype=np.float32) - 0.5 * sq[cn]
    order = np.argsort(-exact, axis=1, kind="stable")[:, :K + 1]
    top = np.take_along_axis(cidx, order, axis=1)            # [N, 9]
    rows = np.arange(N)[:, None]
    selfpos = top == rows
    has_self = selfpos.any(axis=1)
    rem = np.where(has_self, selfpos.argmax(axis=1), K)      # drop self, else 9th
    keep = np.ones((N, K + 1), dtype=bool)
    keep[np.arange(N), rem] = False
    LAST_KNN = top[keep].reshape(N, K)
    return LAST_KNN


def kernel(x, W1, b1, W2, b2):
    x = np.asarray(x, dtype=np.float32)
    W1 = np.asarray(W1, dtype=np.float32)
    b1 = np.asarray(b1, dtype=np.float32)
    W2 = np.asarray(W2, dtype=np.float32)
    b2 = np.asarray(b2, dtype=np.float32)

    xf = x.reshape(N, C)
    knn = _knn_from_device(xf)

    src = np.repeat(np.arange(N, dtype=np.int64), K)
    dst = knn.reshape(-1)
    loops = np.arange(N, dtype=np.int64)
    src = np.concatenate([src, loops])
    dst = np.concatenate([dst, loops])

    deg = np.bincount(dst, minlength=N).astype(np.float32)
    dinv = 1.0 / np.sqrt(np.maximum(deg, 1.0))
    norm = (dinv[src] * dinv[dst]).astype(np.float32)

    try:
        import scipy.sparse as sps
        A = sps.csr_matrix((norm, (dst, src)), shape=(N, N), dtype=np.float32)

        def agg(hw):
            return A @ hw
    except Exception:
        def agg(hw):
            out = np.zeros_like(hw)
            np.add.at(out, dst, hw[src] * norm[:, None])
            return out

    h1 = np.maximum(agg(xf @ W1) + b1, 0.0).astype(np.float32)
    h2 = np.maximum(agg(h1 @ W2) + b2, 0.0).astype(np.float32)
    return h2.reshape(B, H, W, W2.shape[1]).astype(np.float32)


# revision 6
# speedup vs baseline: 2.6166x; 1.0039x over previous
"""Global-KNN GCN kernel for Trainium2 (8 NeuronCores, SPMD).

Device computes the 161-GFLOP pairwise score matrix in fp8 (e4m3,
DoubleRow perf mode: 256-deep contraction per matmul at ~1 cyc/col,
2x bf16 FLOP rate), then per-448-column-chunk top-8 (values + indices)
on the DVE over bf16 scores. The centered -0.5*||x_j||^2 ranking term is
folded into the contraction itself: channels 2046/2047 are sacrificed --
the stationary (row) side carries (1, 1) there and the moving (column)
side carries a coarse+residual fp8 split of the centered norm term. All
of x stays SBUF-resident in fp8 (loaded once, ~12.8 MB/core). Each
core's input is rotated so its own 784-row block sits at column 0; the
stationary row panels are slices of the same resident tensor (except the
last channel group, which has its own modified panel).

Host does the cheap O(N*K) part: merges the 14x8 per-chunk candidates,
exact fp32 re-score of the top-64, builds the KNN edge list, and runs the
two small GCN layers (sparse aggregation + dense matmuls).
"""

import os
import numpy as np

B, H, W, C = 32, 14, 14, 2048
N = B * H * W            # 6272 nodes
K = 8                    # neighbors (excluding self)
N_CORES = 8
ROWS = N // N_CORES      # 784 rows per core
RT = 112                 # rows per tile
NT = ROWS // RT          # 7 row tiles
NB = 448                 # column chunk (psum free size)
NCH = N // NB            # 14 column chunks
KP = C // 256            # 8 channel pair-chunks (256 channels each)
CAND = NCH * 8           # 112 candidates per row
TOPC = 64                # host exact re-score depth

LAST_EXEC_NS = None
LAST_KNN = None
_PROG = None


def _build_program():
    from concourse import bacc, tile, mybir

    f32 = mybir.dt.float32
    bf16 = mybir.dt.bfloat16
    f8 = mybir.dt.float8e4
    u16 = mybir.dt.uint16

    nc = bacc.Bacc("TRN2", target_bir_lowering=False)
    x8 = nc.declare_dram_parameter("x8", [KP, 128, 2, N], f8, isOutput=False)
    xr7d = nc.declare_dram_parameter("xr7", [128, 2, ROWS], f8, isOutput=False)
    cand = nc.declare_dram_parameter("cand", [NT, NCH, RT, 16], u16, isOutput=True)

    Act = mybir.ActivationFunctionType
    DR = mybir.MatmulPerfMode.DoubleRow

    with tile.TileContext(nc) as tc:
        with (
            tc.tile_pool(name="persist", bufs=1) as pp,
            tc.tile_pool(name="score", bufs=6) as cp,
            tc.tile_pool(name="stage", bufs=10) as sp,
            tc.tile_pool(name="psum", bufs=8, space="PSUM") as psp,
        ):
            xs = [pp.tile([128, 2, N], f8, name=f"xs{kp}") for kp in range(KP)]
            HALF = N // 2
            qs = [nc.sync, nc.scalar]
            for h in range(2):
                for kp in range(KP):
                    qs[(h * KP + kp) % 2].dma_start(
                        out=xs[kp][:, :, h * HALF:(h + 1) * HALF],
                        in_=x8[kp, :, :, h * HALF:(h + 1) * HALF])
            xr7 = pp.tile([128, 2, ROWS], f8)
            nc.gpsimd.dma_start(out=xr7[:], in_=xr7d[:])

            for jb in range(2):
                for t in range(NT):
                    r0 = t * RT
                    pss = [
                        psp.tile([RT, NB], f32, tag="ps", name=f"ps_{t}_{jb}_{jp}")
                        for jp in range(7)
                    ]
                    def mm(kp, jp):
                        lhsT = (xs[kp][:, :, r0:r0 + RT] if kp < KP - 1
                                else xr7[:, :, r0:r0 + RT])
                        j = jb * 7 + jp
                        nc.tensor.matmul(
                            pss[jp][:, :],
                            lhsT,
                            xs[kp][:, :, j * NB:(j + 1) * NB],
                            start=(kp == 0), stop=(kp == KP - 1),
                            perf_mode=DR, skip_group_check=True,
                        )
                    if jb == 0 and t < 2:
                        # kp-outer while the x8 halves are still streaming in
                        for kp in range(KP):
                            for jp in range(7):
                                mm(kp, jp)
                    else:
                        # jp-outer: each psum tile completes early in the
                        # block so its top-k drain overlaps the matmuls
                        for jp in range(7):
                            for kp in range(KP):
                                mm(kp, jp)
                    for jp in range(7):
                        j = jb * 7 + jp
                        cb = cp.tile([RT, NB], bf16, tag="cb")
                        nc.scalar.activation(cb[:, :], pss[jp][:, :], Act.Copy)
                        stage = sp.tile([RT, 16], u16, tag="st")
                        nc.vector.max(stage[:, 0:8].bitcast(bf16), cb[:, :])
                        nc.vector.max_index(
                            stage[:, 8:16],
                            stage[:, 0:8].bitcast(bf16),
                            cb[:, :])
                        nc.gpsimd.dma_start(out=cand[t, j], in_=stage[:, :])
    nc.compile()
    return nc


def _knn_from_device(x_flat):
    """Run the SPMD program; return knn [N, K] int64 global indices."""
    global LAST_EXEC_NS, LAST_KNN, _PROG
    import ml_dtypes
    from concourse.bass_utils import run_bass_kernel_spmd

    if _PROG is None:
        _PROG = _build_program()

    xq8 = x_flat.astype(ml_dtypes.float8_e4m3)               # [N, C]
    sq = np.sum(x_flat * x_flat, axis=1, dtype=np.float32)
    nhc = -0.5 * (sq - sq.mean())
    a = nhc.astype(ml_dtypes.float8_e4m3)
    bres = (nhc - a.astype(np.float32)).astype(ml_dtypes.float8_e4m3)
    # x8 layout [kp, p, i, n]: channel = kp*256 + i*128 + p
    x8T = np.ascontiguousarray(xq8.T)                        # [C, N]
    x8 = np.ascontiguousarray(
        x8T.reshape(KP, 2, 128, N).transpose(0, 2, 1, 3))    # [kp, p, i, n]
    # fold the norm term into sacrificed channels 2046/2047 (kp=7, i=1,
    # p=126/127): moving side carries (a, b); stationary side carries (1, 1)
    x8[KP - 1, 126, 1, :] = a
    x8[KP - 1, 127, 1, :] = bres

    one8 = np.float32(1.0).astype(ml_dtypes.float8_e4m3)
    in_maps = []
    for c in range(N_CORES):
        sh = c * ROWS
        x8c = np.ascontiguousarray(np.roll(x8, -sh, axis=3))
        xr7 = np.ascontiguousarray(x8c[KP - 1, :, :, 0:ROWS])
        xr7[126, 1, :] = one8
        xr7[127, 1, :] = one8
        in_maps.append({"x8": x8c, "xr7": xr7})
    res = run_bass_kernel_spmd(
        _PROG, in_maps, list(range(N_CORES)),
        trace=bool(os.environ.get("KNN_TRACE")),
    )
    if res.exec_time_ns is not None:
        LAST_EXEC_NS = res.exec_time_ns

    vals = np.empty((N, CAND), dtype=np.float32)
    cidx = np.empty((N, CAND), dtype=np.int64)
    jbase = (np.arange(NCH, dtype=np.int64) * NB)[None, :, None]
    for c, r in enumerate(res.results):
        o = r["cand"]                                        # [NT, NCH, RT, 16]
        o = o.transpose(0, 2, 1, 3).reshape(ROWS, NCH, 16)
        v = np.ascontiguousarray(o[:, :, 0:8]).view(ml_dtypes.bfloat16)
        loc = o[:, :, 8:16].astype(np.int64)
        gcol = (jbase + loc + c * ROWS) % N
        vals[c * ROWS:(c + 1) * ROWS] = v.astype(np.float32).reshape(ROWS, CAND)
        cidx[c * ROWS:(c + 1) * ROWS] = gcol.reshape(ROWS, CAND)

    # coarse top-TOPC by device score, then exact fp32 re-score
    part = np.argpartition(-vals, TOPC, axis=1)[:, :TOPC]
    cidx = np.take_along_axis(cidx, part, axis=1)            # [N, TOPC]
    exact = np.empty((N, TOPC), dtype=np.float32)
    BLK = 196
    for r0 in range(0, N, BLK):
        r1 = r0 + BLK
        cn = cidx[r0:r1]                                     # [b, TOPC]
        xc = x_flat[cn]                                      # [b, TOPC, C]
        exact[r0:r1] = np.einsum("bc,bkc->bk", x_flat[r0:r1], xc,
                                 dtype=np.float32) - 0.5 * sq[cn]
    order = np.argsort(-exact, axis=1, kind="stable")[:, :K + 1]
    top = np.take_along_axis(cidx, order, axis=1)            # [N, 9]
    rows = np.arange(N)[:, None]
    selfpos = top == rows
    has_self = selfpos.any(axis=1)
    rem = np.where(has_self, selfpos.argmax(axis=1), K)      # drop self, else 9th
    keep = np.ones((N, K + 1), dtype=bool)
    keep[np.arange(N), rem] = False
    LAST_KNN = top[keep].reshape(N, K)
    return LAST_KNN


def kernel(x, W1, b1, W2, b2):
    x = np.asarray(x, dtype=np.float32)
    W1 = np.asarray(W1, dtype=np.float32)
    b1 = np.asarray(b1, dtype=np.float32)
    W2 = np.asarray(W2, dtype=np.float32)
    b2 = np.asarray(b2, dtype=np.float32)

    xf = x.reshape(N, C)
    knn = _knn_from_device(xf)

    src = np.repeat(np.arange(N, dtype=np.int64), K)
    dst = knn.reshape(-1)
    loops = np.arange(N, dtype=np.int64)
    src = np.concatenate([src, loops])
    dst = np.concatenate([dst, loops])

    deg = np.bincount(dst, minlength=N).astype(np.float32)
    dinv = 1.0 / np.sqrt(np.maximum(deg, 1.0))
    norm = (dinv[src] * dinv[dst]).astype(np.float32)

    try:
        import scipy.sparse as sps
        A = sps.csr_matrix((norm, (dst, src)), shape=(N, N), dtype=np.float32)

        def agg(hw):
            return A @ hw
    except Exception:
        def agg(hw):
            out = np.zeros_like(hw)
            np.add.at(out, dst, hw[src] * norm[:, None])
            return out

    h1 = np.maximum(agg(xf @ W1) + b1, 0.0).astype(np.float32)
    h2 = np.maximum(agg(h1 @ W2) + b2, 0.0).astype(np.float32)
    return h2.reshape(B, H, W, W2.shape[1]).astype(np.float32)


# revision 9
# speedup vs baseline: 2.7247x; 1.0413x over previous
"""Global-KNN GCN kernel for Trainium2 (8 NeuronCores, SPMD).

Device computes the 161-GFLOP pairwise score matrix in fp8 (e4m3,
DoubleRow perf mode: 256-deep contraction per matmul at ~1 cyc/col,
2x bf16 FLOP rate), then per-448-column-chunk top-8 (values + indices)
on the DVE over bf16 scores. The centered -0.5*||x_j||^2 ranking term is
folded into the contraction itself: channels 2046/2047 are sacrificed --
the stationary (row) side carries (1, 1) there and the moving (column)
side carries a coarse+residual fp8 split of the centered norm term. All
of x stays SBUF-resident in fp8 (loaded once, ~12.8 MB/core). Each
core's input is rotated so its own 784-row block sits at column 0; the
stationary row panels are slices of the same resident tensor (except the
last channel group, which has its own modified panel).

Host does the cheap O(N*K) part: merges the 14x8 per-chunk candidates,
exact fp32 re-score of the top-64, builds the KNN edge list, and runs the
two small GCN layers (sparse aggregation + dense matmuls).
"""

import os
import numpy as np

B, H, W, C = 32, 14, 14, 2048
N = B * H * W            # 6272 nodes
K = 8                    # neighbors (excluding self)
N_CORES = 8
ROWS = N // N_CORES      # 784 rows per core
RT = 112                 # rows per tile
NT = ROWS // RT          # 7 row tiles
NB = 448                 # column chunk (psum free size)
NCH = N // NB            # 14 column chunks
KP = C // 256            # 8 channel pair-chunks (256 channels each)
CAND = NCH * 8           # 112 candidates per row
TOPC = 64                # host exact re-score depth

LAST_EXEC_NS = None
LAST_KNN = None
_PROG = None


def _build_program():
    from concourse import bacc, tile, mybir

    f32 = mybir.dt.float32
    bf16 = mybir.dt.bfloat16
    f8 = mybir.dt.float8e4
    u16 = mybir.dt.uint16

    nc = bacc.Bacc("TRN2", target_bir_lowering=False)
    x8 = nc.declare_dram_parameter("x8", [KP, 128, 2, N], f8, isOutput=False)
    xr7d = nc.declare_dram_parameter("xr7", [128, 2, ROWS], f8, isOutput=False)
    cand = nc.declare_dram_parameter("cand", [NT, NCH, RT, 16], u16, isOutput=True)

    Act = mybir.ActivationFunctionType
    DR = mybir.MatmulPerfMode.DoubleRow

    with tile.TileContext(nc) as tc:
        with (
            tc.tile_pool(name="persist", bufs=1) as pp,
            tc.tile_pool(name="score", bufs=6) as cp,
            tc.tile_pool(name="stage", bufs=16) as sp,
            tc.tile_pool(name="psum", bufs=8, space="PSUM") as psp,
        ):
            xs = [pp.tile([128, 2, N], f8, name=f"xs{kp}") for kp in range(KP)]
            xr7 = pp.tile([128, 2, ROWS], f8)
            nc.sync.dma_start(out=xr7[:], in_=xr7d[:])
            HALF = N // 2
            for h in range(2):
                for kp in range(KP):
                    nc.scalar.dma_start(
                        out=xs[kp][:, :, h * HALF:(h + 1) * HALF],
                        in_=x8[kp, :, :, h * HALF:(h + 1) * HALF])

            for jb in range(2):
                for t in range(NT):
                    r0 = t * RT
                    pss = [
                        psp.tile([RT, NB], f32, tag="ps", name=f"ps_{t}_{jb}_{jp}")
                        for jp in range(7)
                    ]
                    def mm(kp, jp):
                        lhsT = (xs[kp][:, :, r0:r0 + RT] if kp < KP - 1
                                else xr7[:, :, r0:r0 + RT])
                        j = jb * 7 + jp
                        nc.tensor.matmul(
                            pss[jp][:, :],
                            lhsT,
                            xs[kp][:, :, j * NB:(j + 1) * NB],
                            start=(kp == 0), stop=(kp == KP - 1),
                            perf_mode=DR, skip_group_check=True,
                        )
                    if jb == 0 and t < 2:
                        # kp-outer while the x8 halves are still streaming in
                        for kp in range(KP):
                            for jp in range(7):
                                mm(kp, jp)
                    else:
                        # jp-outer: each psum tile completes early in the
                        # block so its top-k drain overlaps the matmuls
                        for jp in range(7):
                            for kp in range(KP):
                                mm(kp, jp)
                    for jp in range(7):
                        j = jb * 7 + jp
                        cb = cp.tile([RT, NB], bf16, tag="cb")
                        nc.scalar.activation(cb[:, :], pss[jp][:, :], Act.Copy)
                        stage = sp.tile([RT, 16], u16, tag="st")
                        nc.vector.max(stage[:, 0:8].bitcast(bf16), cb[:, :])
                        nc.vector.max_index(
                            stage[:, 8:16],
                            stage[:, 0:8].bitcast(bf16),
                            cb[:, :])
                        nc.sync.dma_start(out=cand[t, j], in_=stage[:, :])
    nc.compile()
    return nc


def _knn_from_device(x_flat):
    """Run the SPMD program; return knn [N, K] int64 global indices."""
    global LAST_EXEC_NS, LAST_KNN, _PROG
    import ml_dtypes
    from concourse.bass_utils import run_bass_kernel_spmd

    if _PROG is None:
        _PROG = _build_program()

    xq8 = x_flat.astype(ml_dtypes.float8_e4m3)               # [N, C]
    sq = np.sum(x_flat * x_flat, axis=1, dtype=np.float32)
    nhc = -0.5 * (sq - sq.mean())
    a = nhc.astype(ml_dtypes.float8_e4m3)
    bres = (nhc - a.astype(np.float32)).astype(ml_dtypes.float8_e4m3)
    # x8 layout [kp, p, i, n]: channel = kp*256 + i*128 + p
    x8T = np.ascontiguousarray(xq8.T)                        # [C, N]
    x8 = np.ascontiguousarray(
        x8T.reshape(KP, 2, 128, N).transpose(0, 2, 1, 3))    # [kp, p, i, n]
    # fold the norm term into sacrificed channels 2046/2047 (kp=7, i=1,
    # p=126/127): moving side carries (a, b); stationary side carries (1, 1)
    x8[KP - 1, 126, 1, :] = a
    x8[KP - 1, 127, 1, :] = bres

    one8 = np.float32(1.0).astype(ml_dtypes.float8_e4m3)
    in_maps = []
    for c in range(N_CORES):
        sh = c * ROWS
        x8c = np.ascontiguousarray(np.roll(x8, -sh, axis=3))
        xr7 = np.ascontiguousarray(x8c[KP - 1, :, :, 0:ROWS])
        xr7[126, 1, :] = one8
        xr7[127, 1, :] = one8
        in_maps.append({"x8": x8c, "xr7": xr7})
    res = run_bass_kernel_spmd(
        _PROG, in_maps, list(range(N_CORES)),
        trace=bool(os.environ.get("KNN_TRACE")),
    )
    if res.exec_time_ns is not None:
        LAST_EXEC_NS = res.exec_time_ns

    vals = np.empty((N, CAND), dtype=np.float32)
    cidx = np.empty((N, CAND), dtype=np.int64)
    jbase = (np.arange(NCH, dtype=np.int64) * NB)[None, :, None]
    for c, r in enumerate(res.results):
        o = r["cand"]                                        # [NT, NCH, RT, 16]
        o = o.transpose(0, 2, 1, 3).reshape(ROWS, NCH, 16)
        v = np.ascontiguousarray(o[:, :, 0:8]).view(ml_dtypes.bfloat16)
        loc = o[:, :, 8:16].astype(np.int64)
        gcol = (jbase + loc + c * ROWS) % N
        vals[c * ROWS:(c + 1) * ROWS] = v.astype(np.float32).reshape(ROWS, CAND)
        cidx[c * ROWS:(c + 1) * ROWS] = gcol.reshape(ROWS, CAND)

    # coarse top-TOPC by device score, then exact fp32 re-score
    part = np.argpartition(-vals, TOPC, axis=1)[:, :TOPC]
    cidx = np.take_along_axis(cidx, part, axis=1)            # [N, TOPC]
    exact = np.empty((N, TOPC), dtype=np.float32)
    BLK = 196
    for r0 in range(0, N, BLK):
        r1 = r0 + BLK
        cn = cidx[r0:r1]                                     # [b, TOPC]
        xc = x_flat[cn]                                      # [b, TOPC, C]
        exact[r0:r1] = np.einsum("bc,bkc->bk", x_flat[r0:r1], xc,
                                 dtype=np.float32) - 0.5 * sq[cn]
    order = np.argsort(-exact, axis=1, kind="stable")[:, :K + 1]
    top = np.take_along_axis(cidx, order, axis=1)            # [N, 9]
    rows = np.arange(N)[:, None]
    selfpos = top == rows
    has_self = selfpos.any(axis=1)
    rem = np.where(has_self, selfpos.argmax(axis=1), K)      # drop self, else 9th
    keep = np.ones((N, K + 1), dtype=bool)
    keep[np.arange(N), rem] = False
    LAST_KNN = top[keep].reshape(N, K)
    return LAST_KNN


def kernel(x, W1, b1, W2, b2):
    x = np.asarray(x, dtype=np.float32)
    W1 = np.asarray(W1, dtype=np.float32)
    b1 = np.asarray(b1, dtype=np.float32)
    W2 = np.asarray(W2, dtype=np.float32)
    b2 = np.asarray(b2, dtype=np.float32)

    xf = x.reshape(N, C)
    knn = _knn_from_device(xf)

    src = np.repeat(np.arange(N, dtype=np.int64), K)
    dst = knn.reshape(-1)
    loops = np.arange(N, dtype=np.int64)
    src = np.concatenate([src, loops])
    dst = np.concatenate([dst, loops])

    deg = np.bincount(dst, minlength=N).astype(np.float32)
    dinv = 1.0 / np.sqrt(np.maximum(deg, 1.0))
    norm = (dinv[src] * dinv[dst]).astype(np.float32)

    try:
        import scipy.sparse as sps
        A = sps.csr_matrix((norm, (dst, src)), shape=(N, N), dtype=np.float32)

        def agg(hw):
            return A @ hw
    except Exception:
        def agg(hw):
            out = np.zeros_like(hw)
            np.add.at(out, dst, hw[src] * norm[:, None])
            return out

    h1 = np.maximum(agg(xf @ W1) + b1, 0.0).astype(np.float32)
    h2 = np.maximum(agg(h1 @ W2) + b2, 0.0).astype(np.float32)
    return h2.reshape(B, H, W, W2.shape[1]).astype(np.float32)


# revision 11
# speedup vs baseline: 2.7577x; 1.0121x over previous
"""Global-KNN GCN kernel for Trainium2 (8 NeuronCores, SPMD).

Device computes the 161-GFLOP pairwise score matrix in fp8 (e4m3,
DoubleRow perf mode: 256-deep contraction per matmul at ~1 cyc/col,
2x bf16 FLOP rate), then per-448-column-chunk top-8 (values + indices)
on the DVE over bf16 scores. The centered -0.5*||x_j||^2 ranking term is
folded into the contraction itself: channels 2046/2047 are sacrificed --
the stationary (row) side carries (1, 1) there and the moving (column)
side carries a coarse+residual fp8 split of the centered norm term. All
of x stays SBUF-resident in fp8 (loaded once, ~12.8 MB/core). Each
core's input is rotated so its own 784-row block sits at column 0; the
stationary row panels are slices of the same resident tensor (except the
last channel group, which has its own modified panel).

Host does the cheap O(N*K) part: merges the 14x8 per-chunk candidates,
exact fp32 re-score of the top-64, builds the KNN edge list, and runs the
two small GCN layers (sparse aggregation + dense matmuls).
"""

import os
import numpy as np

B, H, W, C = 32, 14, 14, 2048
N = B * H * W            # 6272 nodes
K = 8                    # neighbors (excluding self)
N_CORES = 8
ROWS = N // N_CORES      # 784 rows per core
RT = 112                 # rows per tile
NT = ROWS // RT          # 7 row tiles
NB = 448                 # column chunk (psum free size)
NCH = N // NB            # 14 column chunks
KP = C // 256            # 8 channel pair-chunks (256 channels each)
CAND = NCH * 8           # 112 candidates per row
TOPC = 64                # host exact re-score depth

LAST_EXEC_NS = None
LAST_KNN = None
_PROG = None


def _build_program():
    from concourse import bacc, tile, mybir

    f32 = mybir.dt.float32
    bf16 = mybir.dt.bfloat16
    f8 = mybir.dt.float8e4
    u16 = mybir.dt.uint16

    nc = bacc.Bacc("TRN2", target_bir_lowering=False)
    x8 = nc.declare_dram_parameter("x8", [KP, 128, 2, N], f8, isOutput=False)
    xr7d = nc.declare_dram_parameter("xr7", [128, 2, ROWS], f8, isOutput=False)
    cand = nc.declare_dram_parameter("cand", [NT, NCH, RT, 16], u16, isOutput=True)

    Act = mybir.ActivationFunctionType
    DR = mybir.MatmulPerfMode.DoubleRow

    with tile.TileContext(nc) as tc:
        with (
            tc.tile_pool(name="persist", bufs=1) as pp,
            tc.tile_pool(name="score", bufs=6) as cp,
            tc.tile_pool(name="stage", bufs=28) as sp,
            tc.tile_pool(name="psum", bufs=8, space="PSUM") as psp,
        ):
            xs = [pp.tile([128, 2, N], f8, name=f"xs{kp}") for kp in range(KP)]
            xr7 = pp.tile([128, 2, ROWS], f8)
            HALF = N // 2

            def load(kp, h):
                nc.sync.dma_start(
                    out=xs[kp][:, :, h * HALF:(h + 1) * HALF],
                    in_=x8[kp, :, :, h * HALF:(h + 1) * HALF])
            load(0, 0)
            nc.sync.dma_start(out=xr7[:], in_=xr7d[:])
            for kp in range(1, KP):
                load(kp, 0)
            for kp in range(KP):
                load(kp, 1)

            for jb in range(2):
                for t in range(NT):
                    r0 = t * RT
                    pss = [
                        psp.tile([RT, NB], f32, tag="ps", name=f"ps_{t}_{jb}_{jp}")
                        for jp in range(7)
                    ]
                    def mm(kp, jp):
                        lhsT = (xs[kp][:, :, r0:r0 + RT] if kp < KP - 1
                                else xr7[:, :, r0:r0 + RT])
                        j = jb * 7 + jp
                        nc.tensor.matmul(
                            pss[jp][:, :],
                            lhsT,
                            xs[kp][:, :, j * NB:(j + 1) * NB],
                            start=(kp == 0), stop=(kp == KP - 1),
                            perf_mode=DR, skip_group_check=True,
                        )
                    if jb == 0 and t < 2:
                        # kp-outer while the x8 halves are still streaming in
                        for kp in range(KP):
                            for jp in range(7):
                                mm(kp, jp)
                    else:
                        # jp-outer: each psum tile completes early in the
                        # block so its top-k drain overlaps the matmuls
                        for jp in range(7):
                            for kp in range(KP):
                                mm(kp, jp)
                    for jp in range(7):
                        j = jb * 7 + jp
                        cb = cp.tile([RT, NB], bf16, tag="cb")
                        nc.scalar.activation(cb[:, :], pss[jp][:, :], Act.Copy)
                        stage = sp.tile([RT, 16], u16, tag="st")
                        nc.vector.max(stage[:, 0:8].bitcast(bf16), cb[:, :])
                        nc.vector.max_index(
                            stage[:, 8:16],
                            stage[:, 0:8].bitcast(bf16),
                            cb[:, :])
                        nc.sync.dma_start(out=cand[t, j], in_=stage[:, :])
    nc.compile()
    return nc


def _knn_from_device(x_flat):
    """Run the SPMD program; return knn [N, K] int64 global indices."""
    global LAST_EXEC_NS, LAST_KNN, _PROG
    import ml_dtypes
    from concourse.bass_utils import run_bass_kernel_spmd

    if _PROG is None:
        _PROG = _build_program()

    xq8 = x_flat.astype(ml_dtypes.float8_e4m3)               # [N, C]
    sq = np.sum(x_flat * x_flat, axis=1, dtype=np.float32)
    nhc = -0.5 * (sq - sq.mean())
    a = nhc.astype(ml_dtypes.float8_e4m3)
    bres = (nhc - a.astype(np.float32)).astype(ml_dtypes.float8_e4m3)
    # x8 layout [kp, p, i, n]: channel = kp*256 + i*128 + p
    x8T = np.ascontiguousarray(xq8.T)                        # [C, N]
    x8 = np.ascontiguousarray(
        x8T.reshape(KP, 2, 128, N).transpose(0, 2, 1, 3))    # [kp, p, i, n]
    # fold the norm term into sacrificed channels 2046/2047 (kp=7, i=1,
    # p=126/127): moving side carries (a, b); stationary side carries (1, 1)
    x8[KP - 1, 126, 1, :] = a
    x8[KP - 1, 127, 1, :] = bres

    one8 = np.float32(1.0).astype(ml_dtypes.float8_e4m3)
    in_maps = []
    for c in range(N_CORES):
        sh = c * ROWS
        x8c = np.ascontiguousarray(np.roll(x8, -sh, axis=3))
        xr7 = np.ascontiguousarray(x8c[KP - 1, :, :, 0:ROWS])
        xr7[126, 1, :] = one8
        xr7[127, 1, :] = one8
        in_maps.append({"x8": x8c, "xr7": xr7})
    res = run_bass_kernel_spmd(
        _PROG, in_maps, list(range(N_CORES)),
        trace=bool(os.environ.get("KNN_TRACE")),
    )
    if res.exec_time_ns is not None:
        LAST_EXEC_NS = res.exec_time_ns

    vals = np.empty((N, CAND), dtype=np.float32)
    cidx = np.empty((N, CAND), dtype=np.int64)
    jbase = (np.arange(NCH, dtype=np.int64) * NB)[None, :, None]
    for c, r in enumerate(res.results):
        o = r["cand"]                                        # [NT, NCH, RT, 16]
        o = o.transpose(0, 2, 1, 3).reshape(ROWS, NCH, 16)
        v = np.ascontiguousarray(o[:, :, 0:8]).view(ml_dtypes.bfloat16)
        loc = o[:, :, 8:16].astype(np.int64)
        gcol = (jbase + loc + c * ROWS) % N
        vals[c * ROWS:(c + 1) * ROWS] = v.astype(np.float32).reshape(ROWS, CAND)
        cidx[c * ROWS:(c + 1) * ROWS] = gcol.reshape(ROWS, CAND)

    # coarse top-TOPC by device score, then exact fp32 re-score
    part = np.argpartition(-vals, TOPC, axis=1)[:, :TOPC]
    cidx = np.take_along_axis(cidx, part, axis=1)            # [N, TOPC]
    exact = np.empty((N, TOPC), dtype=np.float32)
    BLK = 196
    for r0 in range(0, N, BLK):
        r1 = r0 + BLK
        cn = cidx[r0:r1]                                     # [b, TOPC]
        xc = x_flat[cn]                                      # [b, TOPC, C]
        exact[r0:r1] = np.einsum("bc,bkc->bk", x_flat[r0:r1], xc,
                                 dtype=np.float32) - 0.5 * sq[cn]
    order = np.argsort(-exact, axis=1, kind="stable")[:, :K + 1]
    top = np.take_along_axis(cidx, order, axis=1)            # [N, 9]
    rows = np.arange(N)[:, None]
    selfpos = top == rows
    has_self = selfpos.any(axis=1)
    rem = np.where(has_self, selfpos.argmax(axis=1), K)      # drop self, else 9th
    keep = np.ones((N, K + 1), dtype=bool)
    keep[np.arange(N), rem] = False
    LAST_KNN = top[keep].reshape(N, K)
    return LAST_KNN


def kernel(x, W1, b1, W2, b2):
    x = np.asarray(x, dtype=np.float32)
    W1 = np.asarray(W1, dtype=np.float32)
    b1 = np.asarray(b1, dtype=np.float32)
    W2 = np.asarray(W2, dtype=np.float32)
    b2 = np.asarray(b2, dtype=np.float32)

    xf = x.reshape(N, C)
    knn = _knn_from_device(xf)

    src = np.repeat(np.arange(N, dtype=np.int64), K)
    dst = knn.reshape(-1)
    loops = np.arange(N, dtype=np.int64)
    src = np.concatenate([src, loops])
    dst = np.concatenate([dst, loops])

    deg = np.bincount(dst, minlength=N).astype(np.float32)
    dinv = 1.0 / np.sqrt(np.maximum(deg, 1.0))
    norm = (dinv[src] * dinv[dst]).astype(np.float32)

    try:
        import scipy.sparse as sps
        A = sps.csr_matrix((norm, (dst, src)), shape=(N, N), dtype=np.float32)

        def agg(hw):
            return A @ hw
    except Exception:
        def agg(hw):
            out = np.zeros_like(hw)
            np.add.at(out, dst, hw[src] * norm[:, None])
            return out

    h1 = np.maximum(agg(xf @ W1) + b1, 0.0).astype(np.float32)
    h2 = np.maximum(agg(h1 @ W2) + b2, 0.0).astype(np.float32)
    return h2.reshape(B, H, W, W2.shape[1]).astype(np.float32)
